# revision 1
# baseline (speedup 1.0000x reference)
"""CfC cell (dense MLP) on 8 TRN2 NeuronCores — data-parallel over the batch.

Math (per row r, with x = cat[input, hx]):
    x1   = 1.7159 * tanh(0.666 * (x @ Wb.T + bb))
    ff1  = tanh(x1 @ W1.T + b1)
    ff2  = tanh(x1 @ W2.T + b2)
    t    = sigmoid((x1 @ Wa.T + ba) * ts + (x1 @ Wt.T + bt))
    out  = ff1 + t * (ff2 - ff1)

Device layout: everything runs in "transposed space" — activations are
[features(partitions), rows(free)] so matmuls contract input features on
the partition axis with the weights stationary and activations moving,
and per-feature biases become per-partition ACT bias operands.  The host
pre-transposes inputs to bf16 [384, B/8] per core and un-transposes the
bf16 [256, B/8] output; 0.666 is folded into Wb/bb and 1.7159 into the
four stage-2 weight matrices so no extra scaling ops run on-device.
"""

import numpy as np
import ml_dtypes

import concourse.bass as bass
import concourse.mybir as mybir
import concourse.tile as tile
from concourse.bass_utils import run_bass_kernel_spmd

BF16 = ml_dtypes.bfloat16
_NC_CACHE = {}
_LAST_IN_MAPS = {}
N_CORES = 8
B, IN, H, U = 65536, 128, 256, 256
K = IN + H            # 384 = 3 x 128 contraction chunks for stage 1
R = B // N_CORES      # 8192 rows per core
RC = 1024             # row-chunk processed per iteration (free dim)
NSPLIT = 512          # max PSUM bank free size (f32)

MAX_WAITS = 1         # walrus here rejects instructions with more than one sem wait

# tuning knobs (sim-swept)
KNOBS = {
    "xpool_bufs": 4,
    "apool_bufs": 3,
    "epool_bufs": 4,
    "out_ring": "sync",     # engine issuing output DMAs
    "blend_engine": "vector",  # engine for the final add (non-last chunks)
    "wb_ring": "scalar",    # engine issuing stage-1 weight DMAs
    "tsb_ring": "sync",     # engine issuing the per-chunk ts broadcast
    "xc_split": True,       # 3 per-k input DMAs vs one 3D DMA per chunk
    "lookahead": 1,         # stage-1 chunks emitted ahead of stage-2
    "ab_first": False,      # tanh-head matmuls emitted before the sigmoid
                            # head: the ab group's long epilogue chain
                            # (stt+sigmoid) then overlaps the ff tanh work
                            # instead of blocking it (HW A/B: -3.6us)
    "taper": 512,           # 2x512 tail chunks shorten the exposed
                            # end-of-pipeline epilogue chain (HW A/B: -0.9us)
    "alt_rings": False,     # alternate input loads between SP and ACT rings
    "tsb_mode": "dma",      # ts broadcast: DMA replication vs gpsimd on-chip
    "tail_split": False,    # halve the last chunk epilogue ops (net loss in sim)
    "ps1_bufs": 1,
    "ps2_bufs": 3,
}


def _spill_excess_waits(nc, max_waits=MAX_WAITS):
    """walrus in this container fails codegen for instructions carrying
    more than a couple of sem waits ("Too many sync wait commands").
    Move the excess onto same-engine nops inserted just before the
    instruction; engines execute a block's instructions in order, so the
    wait semantics are unchanged."""
    for f in nc.m.functions:
        for bb in f.blocks:
            insts = bb.instructions
            i = 0
            while i < len(insts):
                inst = insts[i]
                si = inst.sync_info
                if si is not None and si.on_wait and len(si.on_wait) > max_waits:
                    waits = list(si.on_wait)
                    keep = waits[-max_waits:]
                    spill = waits[:-max_waits]
                    while spill:
                        chunk = spill[:max_waits]
                        spill = spill[max_waits:]
                        nop = mybir.InstNoOp(
                            name=nc.get_next_instruction_name(),
                            text_hint="wait_spill",
                            engine=inst.engine,
                        )
                        nop.sync_info = mybir.SyncInfo(on_wait=chunk, on_update=[])
                        insts.insert(i, nop)
                        i += 1
                    inst.sync_info = mybir.SyncInfo(
                        on_wait=keep, on_update=list(si.on_update or [])
                    )
                i += 1


def _build_nc(repeat=1):
    AF = mybir.ActivationFunctionType
    ALU = mybir.AluOpType
    f32 = mybir.dt.float32
    bf = mybir.dt.bfloat16

    nc = bass.Bass()
    xT = nc.declare_dram_parameter("xT", [K, R], bf, isOutput=False)
    ts = nc.declare_dram_parameter("ts", [1, R], bf, isOutput=False)
    wb = nc.declare_dram_parameter("wb", [128, 3, U], bf, isOutput=False)
    bbp = nc.declare_dram_parameter("bb", [128, 2], f32, isOutput=False)
    w2 = nc.declare_dram_parameter("w2", [128, 4, 2, H], bf, isOutput=False)
    b2p = nc.declare_dram_parameter("b2", [128, 4, 2], f32, isOutput=False)
    outT = nc.declare_dram_parameter("outT", [H, R], bf, isOutput=True)

    with (
        tile.TileContext(nc) as tc,
        tc.tile_pool(name="w", bufs=1) as wpool,
        tc.tile_pool(name="x", bufs=KNOBS["xpool_bufs"]) as xpool,
        tc.tile_pool(name="act", bufs=KNOBS["apool_bufs"]) as apool,
        tc.tile_pool(name="ew", bufs=KNOBS["epool_bufs"]) as epool,
        tc.tile_pool(name="ps1", bufs=KNOBS["ps1_bufs"], space="PSUM") as ps1,
        tc.tile_pool(name="ps2", bufs=KNOBS["ps2_bufs"], space="PSUM") as ps2,
    ):
        wb_eng = getattr(nc, KNOBS["wb_ring"])
        wbt = wpool.tile([128, 3, U], bf)
        for _k in range(3):
            wb_eng.dma_start(out=wbt[:, _k, :], in_=wb[:, _k, :])
        bbt = wpool.tile([128, 2], f32)
        wb_eng.dma_start(out=bbt, in_=bbp[:, :])
        # stage-2 weights go on the gpsimd ring: off the SP ring so they
        # don't delay the first input chunk, and off the ACT queue so its
        # descriptor work never delays activations (HW A/B: -0.7us)
        w2t = wpool.tile([128, 4, 2, H], bf)
        nc.gpsimd.dma_start(out=w2t, in_=w2[:, :, :, :])
        b2t = wpool.tile([128, 4, 2], f32)
        nc.gpsimd.dma_start(out=b2t, in_=b2p[:, :, :])

        xT3 = xT[:, :].rearrange("(k p) r -> p k r", p=128)

        ts_row = None
        if KNOBS["tsb_mode"] == "pool":
            ts_row = wpool.tile([1, R], bf)
            nc.sync.dma_start(out=ts_row, in_=ts[0:1, :])

        # PE prewarm: ~4us of dummy matmuls during the initial DMA window
        # releases the HAM clock gate (1.2 -> 2.4 GHz) before real work.
        warm = wpool.tile([128, NSPLIT], bf)
        nc.vector.memset(warm, 0.0)
        wps = ps1.tile([128, NSPLIT], f32, tag="p1")
        for _i in range(10):
            nc.tensor.matmul(
                wps, lhsT=warm[:, :128], rhs=warm, start=True, stop=True
            )

        import contextlib

        loop_cm = (
            tc.For_i(
                0, repeat, 1,
                hint_engines=(
                    mybir.EngineType.PE,
                    mybir.EngineType.Activation,
                    mybir.EngineType.DVE,
                    mybir.EngineType.SP,
                ),
            )
            if repeat > 1
            else contextlib.nullcontext()
        )
        with loop_cm:
            _emit_body(
                nc, tc, AF, ALU, f32, bf,
                xpool, apool, epool, ps1, ps2,
                xT3, ts, outT, wbt, w2t, bbt, b2t, ts_row,
            )

    _spill_excess_waits(nc)
    return nc


def _emit_body(
    nc, tc, AF, ALU, f32, bf,
    xpool, apool, epool, ps1, ps2,
    xT3, ts, outT, wbt, w2t, bbt, b2t, ts_row=None,
):
    def chunk_load(c):
        start, rc = c
        sl = slice(start, start + rc)
        ld_eng = (
            (nc.sync if (start // RC) % 2 == 0 else nc.scalar)
            if KNOBS["alt_rings"] else nc.sync
        )
        if KNOBS["xc_split"]:
            xc = []
            for k in range(3):
                xk = xpool.tile([128, rc], bf, tag=f"xc{k}")
                ld_eng.dma_start(out=xk, in_=xT3[:, k, sl])
                xc.append(xk)
        else:
            x3 = xpool.tile([128, 3, rc], bf, tag="xc")
            nc.sync.dma_start(out=x3, in_=xT3[:, :, sl])
            xc = [x3[:, k, :] for k in range(3)]
        tsb = xpool.tile([128, rc], bf, tag="tsb")
        if KNOBS["tsb_mode"] == "pool":
            nc.gpsimd.partition_broadcast(tsb, ts_row[0:1, sl])
        else:
            getattr(nc, KNOBS["tsb_ring"]).dma_start(
                out=tsb, in_=ts[0:1, sl].partition_broadcast(128)
            )
        return xc, tsb

    def stage1(c, xc, tsb):
        start, rc = c
        # x1T[u] = tanh(sum_k wb[k,u].T @ xT[k] + bb[u])
        xp = apool.tile([128, 2, rc], bf, tag="xp")
        for u in range(2):
            p1 = ps1.tile([128, rc], f32, tag="p1")
            for n0 in range(0, rc, NSPLIT):
                for k in range(3):
                    nc.tensor.matmul(
                        p1[:, n0 : n0 + NSPLIT],
                        lhsT=wbt[:, k, u * 128 : (u + 1) * 128],
                        rhs=xc[k][:, n0 : n0 + NSPLIT],
                        start=(k == 0),
                        stop=(k == 2),
                    )
            nc.scalar.activation(
                out=xp[:, u, :], in_=p1, func=AF.Tanh,
                bias=bbt[:, u : u + 1],
            )
        # x'' = x' * ts so the sigmoid argument becomes one accumulated
        # matmul group:  Wa@x'' + Wt@x'  (+ ba*ts via stt, + bt via ACT)
        xpp = apool.tile([128, 2, rc], bf, tag="xpp")
        for u in range(2):
            nc.vector.tensor_mul(out=xpp[:, u, :], in0=xp[:, u, :], in1=tsb)
        return xp, xpp, tsb

    def stage2(c, xp, xpp, tsb, last=False):
        start, rc = c
        sl = slice(start, start + rc)
        # heads j: 0=ff1 1=ff2 2=a 3=b
        for h in range(2):
            def head_matmul(dst, srcs):
                for n0 in range(0, rc, NSPLIT):
                    nmm = 2 * len(srcs)
                    i = 0
                    for j, src in srcs:
                        for k in range(2):
                            nc.tensor.matmul(
                                dst[:, n0 : n0 + NSPLIT],
                                lhsT=w2t[:, j, k, h * 128 : (h + 1) * 128],
                                rhs=src[:, k, n0 : n0 + NSPLIT],
                                start=(i == 0),
                                stop=(i == nmm - 1),
                            )
                            i += 1

            # on the last chunk, run the epilogue per n-half so the tail
            # chain overlaps the still-running second-half matmuls
            sw = NSPLIT if (last and KNOBS["tail_split"]) else rc
            splits = [slice(n0, n0 + sw) for n0 in range(0, rc, sw)]
            if KNOBS["ab_first"]:
                ab = ps2.tile([128, rc], f32, tag="ps2")
                head_matmul(ab, [(2, xpp), (3, xp)])
            z1 = epool.tile([128, rc], f32, tag="z1")
            tt = epool.tile([128, rc], bf, tag="tt")

            def sigmoid_chain():
                for ns in splits:
                    nc.vector.scalar_tensor_tensor(
                        out=z1[:, ns], in0=tsb[:, ns],
                        scalar=b2t[:, 2, h : h + 1], in1=ab[:, ns],
                        op0=ALU.mult, op1=ALU.add,
                    )
                    nc.scalar.activation(
                        out=tt[:, ns], in_=z1[:, ns], func=AF.Sigmoid,
                        bias=b2t[:, 3, h : h + 1],
                    )

            if KNOBS["ab_first"]:
                sigmoid_chain()
            pf1 = ps2.tile([128, rc], f32, tag="ps2")
            head_matmul(pf1, [(0, xp)])
            pf2 = ps2.tile([128, rc], f32, tag="ps2")
            head_matmul(pf2, [(1, xp)])
            if not KNOBS["ab_first"]:
                ab = ps2.tile([128, rc], f32, tag="ps2")
                head_matmul(ab, [(2, xpp), (3, xp)])
                sigmoid_chain()
            f1 = epool.tile([128, rc], bf, tag="f1")
            f2 = epool.tile([128, rc], bf, tag="f2")
            d = epool.tile([128, rc], bf, tag="d")
            m = epool.tile([128, rc], bf, tag="m")
            o = epool.tile([128, rc], bf, tag="o")
            out_eng = getattr(nc, KNOBS["out_ring"])
            for ns in splits:
                nc.scalar.activation(
                    out=f1[:, ns], in_=pf1[:, ns], func=AF.Tanh,
                    bias=b2t[:, 0, h : h + 1],
                )
                nc.scalar.activation(
                    out=f2[:, ns], in_=pf2[:, ns], func=AF.Tanh,
                    bias=b2t[:, 1, h : h + 1],
                )
                nc.vector.tensor_sub(out=d[:, ns], in0=f2[:, ns], in1=f1[:, ns])
                nc.vector.tensor_mul(out=m[:, ns], in0=tt[:, ns], in1=d[:, ns])
                if last or KNOBS["blend_engine"] == "vector":
                    nc.vector.tensor_add(out=o[:, ns], in0=f1[:, ns], in1=m[:, ns])
                else:
                    nc.gpsimd.tensor_add(out=o[:, ns], in0=f1[:, ns], in1=m[:, ns])
                nc.sync.dma_start(
                    out=outT[h * 128 : (h + 1) * 128, sl][:, ns], in_=o[:, ns]
                )

    # software-pipelined emission: stage 1 of chunk c+1 is emitted before
    # stage 2 of chunk c, so the in-order ACT/PE queues always have the next
    # chunk's independent work ahead of the current chunk's epilogue chains.
    chunks = []
    pos = 0
    while pos < R - KNOBS["taper"] * 2:
        chunks.append((pos, RC))
        pos += RC
    while pos < R:
        chunks.append((pos, KNOBS["taper"]))
        pos += KNOBS["taper"]
    DEPTH = KNOBS["lookahead"]
    pending = [(c, stage1(c, *chunk_load(c))) for c in chunks[:DEPTH]]
    for c in chunks[DEPTH:]:
        pending.append((c, stage1(c, *chunk_load(c))))
        pc, ps = pending.pop(0)
        stage2(pc, *ps, last=(pc[0] + pc[1] >= R - 2 * KNOBS["taper"]))
    for pc, ps in pending:
        stage2(pc, *ps, last=True)


def kernel(input, hx, ts, Wb, bb, W1, b1, W2, b2, Wa, ba, Wt, bt):
    input = np.asarray(input)
    hx = np.asarray(hx)
    ts = np.asarray(ts)

    # host-side weight prep (shared across cores)
    wb_h = np.ascontiguousarray(
        (0.666 * np.asarray(Wb)).T.reshape(3, 128, U).transpose(1, 0, 2)
    ).astype(BF16)
    bb_h = np.ascontiguousarray(
        (0.666 * np.asarray(bb)).reshape(2, 128).T
    ).astype(np.float32)
    w2_h = np.ascontiguousarray(
        np.stack(
            [
                (1.7159 * np.asarray(W)).T.reshape(2, 128, H).transpose(1, 0, 2)
                for W in (W1, W2, Wa, Wt)
            ],
            axis=1,
        )
    ).astype(BF16)  # [128, 4(j), 2(k), H]
    b2_h = np.ascontiguousarray(
        np.stack(
            [np.asarray(b).reshape(2, 128).T for b in (b1, b2, ba, bt)], axis=1
        )
    ).astype(np.float32)  # [128, 4(j), 2(h)]

    xT_full = np.concatenate([input, hx], axis=1).T.astype(BF16)  # [384, B]
    ts_full = ts.reshape(1, B).astype(BF16)

    if "nc" not in _NC_CACHE:
        _NC_CACHE["nc"] = _build_nc()
    nc = _NC_CACHE["nc"]

    in_maps = []
    for c in range(N_CORES):
        sl = slice(c * R, (c + 1) * R)
        in_maps.append(
            {
                "xT": np.ascontiguousarray(xT_full[:, sl]),
                "ts": np.ascontiguousarray(ts_full[:, sl]),
                "wb": wb_h,
                "bb": bb_h,
                "w2": w2_h,
                "b2": b2_h,
            }
        )

    _LAST_IN_MAPS["maps"] = in_maps
    res = run_bass_kernel_spmd(nc, in_maps, core_ids=list(range(N_CORES)))

    out = np.empty((B, H), np.float32)
    for c in range(N_CORES):
        out[c * R : (c + 1) * R, :] = res.results[c]["outT"].T.astype(np.float32)
    return out


# ---------------------------------------------------------------------------
# Timing support (used by test.py; the grading harness only calls kernel()).
# No NTFF profiling hook is available under axon in this container, so we
# estimate device time as wall-clock of the jitted SPMD execution (inputs
# pre-placed on device) minus the same measurement for a trivial kernel.
# ---------------------------------------------------------------------------

def _make_runner(nc, in_maps):
    import jax
    from jax.sharding import Mesh, PartitionSpec, NamedSharding
    from jax.experimental.shard_map import shard_map
    from concourse import bass2jax

    bass2jax.install_neuronx_cc_hook()
    n_cores = len(in_maps)

    in_names, out_names, out_avals, zero_outs = [], [], [], []
    partition_name = nc.partition_id_tensor.name if nc.partition_id_tensor else None
    for alloc in nc.m.functions[0].allocations:
        if not isinstance(alloc, mybir.MemoryLocationSet):
            continue
        name = alloc.memorylocations[0].name
        if alloc.kind == "ExternalInput":
            if name != partition_name:
                in_names.append(name)
        elif alloc.kind == "ExternalOutput":
            out_names.append(name)
            shape = tuple(alloc.tensor_shape)
            dtype = mybir.dt.np(alloc.dtype)
            out_avals.append(jax.core.ShapedArray(shape, dtype))
            zero_outs.append(np.zeros(shape, dtype))
    n_params = len(in_names)
    in_names = in_names + out_names
    if partition_name is not None:
        in_names.append(partition_name)

    def _body(*args):
        operands = list(args)
        if partition_name is not None:
            operands.append(bass2jax.partition_id_tensor())
        outs = bass2jax._bass_exec_p.bind(
            *operands,
            out_avals=tuple(out_avals),
            in_names=tuple(in_names),
            out_names=tuple(out_names),
            lowering_input_output_aliases=(),
            sim_require_finite=True,
            sim_require_nnan=True,
            nc=nc,
        )
        return tuple(outs)

    devices = jax.devices()[:n_cores]
    mesh = Mesh(np.asarray(devices), ("core",))
    spec = PartitionSpec("core")
    sharded = jax.jit(
        shard_map(
            _body,
            mesh=mesh,
            in_specs=(spec,) * (n_params + len(out_names)),
            out_specs=(spec,) * len(out_names),
            check_rep=False,
        ),
        keep_unused=True,
    )
    sh = NamedSharding(mesh, spec)
    dev_args = [
        jax.device_put(
            np.concatenate([np.asarray(m[k]) for m in in_maps], axis=0), sh
        )
        for k in in_names[:n_params]
    ] + [
        jax.device_put(
            np.zeros((n_cores * z.shape[0], *z.shape[1:]), z.dtype), sh
        )
        for z in zero_outs
    ]

    def run():
        return sharded(*dev_args)

    return run


def _build_tiny_nc():
    """Minimal kernel, used to measure fixed dispatch overhead."""
    nc = bass.Bass()
    x = nc.declare_dram_parameter("x", [128, 128], mybir.dt.float32, isOutput=False)
    y = nc.declare_dram_parameter("y", [128, 128], mybir.dt.float32, isOutput=True)
    with tile.TileContext(nc) as tc, tc.tile_pool(name="p", bufs=1) as pool:
        t = pool.tile([128, 128], mybir.dt.float32)
        nc.sync.dma_start(out=t, in_=x[:, :])
        nc.sync.dma_start(out=y[:, :], in_=t)
    _spill_excess_waits(nc)
    return nc


def measure_exec_ns(in_maps=None, reps=10, lo_repeat=200, hi_repeat=500):
    """Best-effort HW time via repeat-scaling: the kernel body is run in a
    hardware For_i loop `lo_repeat` and `hi_repeat` times in two NEFFs;
    per-pass device time is the slope (wall[hi] - wall[lo]) / (hi - lo),
    which cancels the large (tens of ms, drifting) axon dispatch overhead.
    Mins over interleaved reps reject scheduling noise on the shared
    terminal."""
    import time
    import jax

    if in_maps is None:
        in_maps = _LAST_IN_MAPS["maps"]
    runs = {}
    for rep in (lo_repeat, hi_repeat):
        runs[rep] = _make_runner(_build_nc(repeat=rep), in_maps)
        jax.block_until_ready(runs[rep]())
    mins = {rep: float("inf") for rep in runs}
    for _ in range(reps):
        for rep in runs:
            t0 = time.perf_counter()
            jax.block_until_ready(runs[rep]())
            t1 = time.perf_counter()
            mins[rep] = min(mins[rep], t1 - t0)
    ns = max(0.0, mins[hi_repeat] - mins[lo_repeat]) * 1e9 / (hi_repeat - lo_repeat)
    print(
        f"[timing] min wall x{lo_repeat} {mins[lo_repeat] * 1e3:.1f} ms, "
        f"x{hi_repeat} {mins[hi_repeat] * 1e3:.1f} ms "
        f"-> est HW {ns:.0f} ns/pass"
    )
    return int(ns)


